# revision 1
# baseline (speedup 1.0000x reference)
"""Trainium2 Bass kernel for nn_CausalSelfAttention_14877766713804.

Full causal self-attention block (QKV proj + rmsnorm + rope + causal SDPA
with value-embedding mix + output proj), distributed over 8 NeuronCores as
(batch, head-group): core c handles batch c//2 and heads (c%2)*4..(c%2)*4+4.
The only cross-core exchange is a pair AllGather of the attention output
(feature-transposed) before the column-sharded output projection.

Self-contained: hardcodes shapes from the problem spec.
"""
import numpy as np

import concourse.bacc as bacc
import concourse.mybir as mybir
import concourse.tile as tile
from concourse.masks import make_identity
from concourse.bass_utils import run_bass_kernel_spmd

dt = mybir.dt
AF = mybir.ActivationFunctionType
ALU = mybir.AluOpType

# Problem constants
B, T, DIM, H, HD = 4, 2048, 1024, 8, 128
HDIM = H * HD                     # 1024
ATTN_SCALE = 0.12
EPS = 1.1920929e-07               # np.finfo(np.float32).eps

N_CORES = 8
HG = 4                            # heads per core
F = HG * HD                       # 512 local qkv features per section
P = 128
STRIPE = 512                      # q-stripe width in attention
NEG_INF = -1.0e30


def _build_nc(t_len=T, phases=3, reps_ph12=1, reps_tail=1):
    """Build + compile the SPMD single-core program (same on all 8 cores).

    reps_ph12 > 1 wraps phases 1+2 in a hardware loop; reps_tail > 1
    python-unrolls the collective+projection tail. Both are only for
    execution-time measurement (For_i around a collective desyncs).
    """
    from contextlib import ExitStack
    n_tt = t_len // P             # token tiles
    n_st = t_len // STRIPE        # q stripes
    nc = bacc.Bacc(None, target_bir_lowering=False, num_devices=N_CORES)

    # ---- external I/O (per-core shards, host-prepped layouts) ----
    x_d = nc.dram_tensor("x", [t_len, DIM], dt.float32, kind="ExternalInput")
    wq_d = nc.dram_tensor("wq", [P, 8, 3 * F], dt.float32, kind="ExternalInput")
    ve_d = nc.dram_tensor("ve", [t_len, F], dt.float32, kind="ExternalInput")
    lam_d = nc.dram_tensor("lam", [P, 2], dt.float32, kind="ExternalInput")
    cos_d = nc.dram_tensor("cosr", [P, n_tt, HD], dt.float32, kind="ExternalInput")
    sin_d = nc.dram_tensor("sinr", [P, n_tt, HD], dt.float32, kind="ExternalInput")
    msk_d = nc.dram_tensor("dmask", [P, P], dt.float32, kind="ExternalInput")
    wp_d = nc.dram_tensor("wp", [P, 8, F], dt.float32, kind="ExternalInput")
    bias_d = nc.dram_tensor("bias", [1, F], dt.float32, kind="ExternalInput")
    out_d = nc.dram_tensor("out", [t_len, F], dt.float32, kind="ExternalOutput")

    with tile.TileContext(nc) as tc:
        with (
            tc.tile_pool(name="const", bufs=1) as const,
            tc.tile_pool(name="dram", bufs=1, space="DRAM") as dram,
            tc.tile_pool(name="big", bufs=1) as big,
        ):
            # ---- constants ----
            ident = const.tile([P, P], dt.float32)
            make_identity(nc, ident)
            ident_bf = const.tile([P, P], dt.bfloat16)
            nc.vector.tensor_copy(ident_bf[:], ident[:])
            mask_sb = const.tile([P, P], dt.float32)
            lam_sb = const.tile([P, 2], dt.float32)
            cos_sb = const.tile([P, n_tt, HD], dt.float32)
            sin_sb = const.tile([P, n_tt, HD], dt.float32)
            ones_bf = const.tile([1, P], dt.bfloat16)
            nc.vector.memset(ones_bf[:], 1.0)
            bias_bf = const.tile([1, F], dt.bfloat16)
            bias_f32 = const.tile([1, F], dt.float32)

            rep_loop = ExitStack()
            if reps_ph12 > 1:
                rep_loop.enter_context(tc.For_i(0, reps_ph12, 1))

            # ---- persistent big tensors ----
            QT = big.tile([P, HG, t_len], dt.bfloat16)   # [hd, h, t] roped q
            KT = big.tile([P, HG, t_len], dt.bfloat16)
            Vb = big.tile([P, n_tt, HG * (HD + 1)], dt.bfloat16)  # +ones col
            nc.vector.memset(Vb[:], 1.0)  # ones cols; v blocks overwritten
            # per-head transposed-y exchange buffers (bf16): one pair
            # AllGather per head, overlapped with later heads' attention
            ytl_ds = [dram.tile([P, t_len], dt.bfloat16, name=f"ytl{h}")
                      for h in range(HG)]
            ytg_ds = [dram.tile([2 * P, t_len], dt.bfloat16, name=f"ytg{h}")
                      for h in range(HG)]

            # ============ phase 1: qkv + norm + rope + transposes ============
            with (
                tc.tile_pool(name="wt", bufs=1) as wtp,
                tc.tile_pool(name="stage", bufs=2) as stage,
                tc.tile_pool(name="xin", bufs=2) as xin,
                tc.tile_pool(name="xtp", bufs=3) as xtp,
                tc.tile_pool(name="qk", bufs=3) as qkp,
                tc.tile_pool(name="vesb", bufs=3) as vesb,
                tc.tile_pool(name="small", bufs=6) as small,
                tc.tile_pool(name="rtmp", bufs=4) as rtmpp,
                tc.tile_pool(name="tps", bufs=2, space="PSUM") as tps,
                tc.tile_pool(name="mmps", bufs=4, space="PSUM") as mmps,
            ):
                # stage + round the qkv weights to f32r
                WT = wtp.tile([P, 8, 3 * F], dt.float32r)
                for dc in range(8):
                    for fh in range(2):
                        wstg = stage.tile([P, 3 * F // 2], dt.float32,
                                          name="wstg", tag="wstg")
                        nc.scalar.dma_start(
                            wstg[:], wq_d[:, dc, fh * 768:(fh + 1) * 768])
                        nc.vector.tensor_copy(
                            WT[:, dc, fh * 768:(fh + 1) * 768], wstg[:])
                # const tables (needed from the first rope onwards)
                nc.scalar.dma_start(lam_sb[:], lam_d[:])
                nc.scalar.dma_start(cos_sb[:], cos_d[:])
                nc.scalar.dma_start(sin_sb[:], sin_d[:])
                nc.scalar.dma_start(mask_sb[:], msk_d[:])
                nc.scalar.dma_start(bias_f32[:], bias_d[:])
                nc.vector.tensor_copy(bias_bf[:], bias_f32[:])

                # software pipeline: stage A (load/transpose/matmul/evict/
                # rstd) for tile tt, stage B (rope + qk transposes) for tile
                # tt-1 — so PE never stalls on the rope chain mid-queue.
                pending = {}

                def stage_b(tt):
                    q_t, k_t, rstd = pending.pop(tt)
                    cos_t = cos_sb[:, tt, :]
                    sin_t = sin_sb[:, tt, :]
                    HH = HD // 2
                    for qk in range(2):
                        src = q_t if qk == 0 else k_t
                        eng = nc.vector
                        add_eng = nc.gpsimd
                        for h in range(HG):
                            rs = rstd[:, qk * HG + h:qk * HG + h + 1]
                            hsl = src[:, h * HD:(h + 1) * HD]
                            r1 = rtmpp.tile([P, HD], dt.bfloat16, name="r1", tag="r1")
                            eng.scalar_tensor_tensor(
                                out=r1[:, 0:HH], in0=hsl[:, HH:HD],
                                scalar=rs, in1=sin_t[:, 0:HH],
                                op0=ALU.mult, op1=ALU.mult,
                            )
                            eng.scalar_tensor_tensor(
                                out=r1[:, HH:HD], in0=hsl[:, 0:HH],
                                scalar=rs, in1=sin_t[:, HH:HD],
                                op0=ALU.mult, op1=ALU.mult,
                            )
                            r2 = rtmpp.tile([P, HD], dt.bfloat16, name="r2", tag="r2")
                            eng.scalar_tensor_tensor(
                                out=r2[:], in0=hsl, scalar=rs, in1=cos_t,
                                op0=ALU.mult, op1=ALU.mult,
                            )
                            add_eng.tensor_add(hsl, r1[:], r2[:])
                            ps = tps.tile([P, P], dt.bfloat16, name="ps_qk", tag="tpb")
                            nc.tensor.transpose(ps[:], hsl, ident_bf[:])
                            dstT = QT if qk == 0 else KT
                            if h % 2 == 0:
                                nc.scalar.copy(dstT[:, h, tt * P:(tt + 1) * P], ps[:])
                            else:
                                nc.vector.tensor_copy(dstT[:, h, tt * P:(tt + 1) * P], ps[:])

                for tt in range(n_tt):
                    x_t = xin.tile([P, DIM], dt.float32, name="x_t", tag="x")
                    nc.sync.dma_start(x_t[:], x_d[tt * P:(tt + 1) * P, :])
                    # transpose x tile -> [d, t] chunks (rounded to f32r)
                    xT = xtp.tile([P, 8, P], dt.float32r, name="xT", tag="xT")
                    for dc in range(8):
                        ps = tps.tile([P, P], dt.float32, name="ps_xt", tag="tp")
                        nc.tensor.transpose(ps[:], x_t[:, dc * P:(dc + 1) * P], ident[:])
                        nc.scalar.copy(xT[:, dc, :], ps[:])

                    ve_t = vesb.tile([P, F], dt.float32, name="ve_t", tag="ve")
                    nc.sync.dma_start(ve_t[:], ve_d[tt * P:(tt + 1) * P, :])
                    nc.vector.tensor_scalar_mul(ve_t[:], ve_t[:], lam_sb[:, 1:2])

                    q_t = qkp.tile([P, F], dt.bfloat16, name="q_t", tag="q")
                    k_t = qkp.tile([P, F], dt.bfloat16, name="k_t", tag="k")
                    qss = small.tile([P, 2 * HG], dt.float32, name="qss", tag="qss")
                    scr = rtmpp.tile([P, F], dt.float32, name="scr", tag="scr")

                    for grp in range(3):
                        mm = mmps.tile([P, F], dt.float32, name="mm", tag="mm")
                        for dc in range(8):
                            nc.tensor.matmul(
                                mm[:], xT[:, dc, :],
                                WT[:, dc, grp * F:(grp + 1) * F],
                                start=(dc == 0), stop=(dc == 7),
                            )
                        if grp == 0:
                            nc.scalar.copy(q_t[:], mm[:])
                        elif grp == 1:
                            nc.scalar.copy(k_t[:], mm[:])
                        else:
                            # v = lam0 * v + lam1 * ve, into bf16 V with ones col
                            vdst = Vb[:, tt, :].rearrange("p (h c) -> p h c", h=HG)
                            nc.vector.scalar_tensor_tensor(
                                out=vdst[:, :, 0:HD],
                                in0=mm[:].rearrange("p (h c) -> p h c", h=HG),
                                scalar=lam_sb[:, 0:1],
                                in1=ve_t[:].rearrange("p (h c) -> p h c", h=HG),
                                op0=ALU.mult, op1=ALU.add,
                            )

                    # rstd = 1/sqrt(mean(q^2) + eps) per head
                    for i, src in enumerate((q_t, k_t)):
                        nc.gpsimd.tensor_mul(scr[:], src[:], src[:])
                        nc.vector.tensor_reduce(
                            qss[:, i * HG:(i + 1) * HG],
                            scr[:].rearrange("p (h c) -> p h c", h=HG),
                            axis=mybir.AxisListType.X, op=ALU.add)
                    rstd = small.tile([P, 2 * HG], dt.float32, name="rstd", tag="rstd")
                    nc.vector.tensor_scalar_add(rstd[:], qss[:], HD * EPS)
                    nc.scalar.activation(rstd[:], rstd[:], AF.Sqrt,
                                         scale=1.0 / HD, bias=0.0)
                    nc.vector.reciprocal(rstd[:], rstd[:])

                    pending[tt] = (q_t, k_t, rstd)
                    if tt >= 1:
                        stage_b(tt - 1)
                stage_b(n_tt - 1)

            # ============ phase 2: causal attention per head ============
            if phases < 2:
                with tc.tile_pool(name="dbg", bufs=1) as dbgp:
                    z = dbgp.tile([P, F], dt.float32r)
                    nc.vector.tensor_copy(z[:], QT[:, 0, 0:F])
                    for tt in range(n_tt):
                        nc.sync.dma_start(
                            out_d[tt * P:(tt + 1) * P, :].bitcast(dt.float32r), z[:])
            if phases >= 2:
                with (
                    tc.tile_pool(name="Y", bufs=1) as yp,
                    tc.tile_pool(name="pt", bufs=2 + 4 * n_st) as ptp,
                    tc.tile_pool(name="ytst", bufs=2) as ytstp,
                    tc.tile_pool(name="rec", bufs=4) as recp,
                    tc.tile_pool(name="sps", bufs=2, space="PSUM") as sps,
                    tc.tile_pool(name="yps", bufs=4, space="PSUM") as yps,
                    tc.tile_pool(name="ytps", bufs=2, space="PSUM") as ytps,
                ):
                    Y = yp.tile([P, n_tt, F], dt.float32)
                    for h in range(HG):
                        for s in range(n_st):
                            kmax = 4 * s + 3      # last k tile of this stripe
                            pts = []
                            for kt in range(kmax + 1):
                                qoff = max(0, (kt - 4 * s) * P)
                                sp = sps.tile([P, STRIPE], dt.float32,
                                              name="sp", tag="sp")
                                nc.tensor.matmul(
                                    sp[:, qoff:STRIPE],
                                    KT[:, h, kt * P:(kt + 1) * P],
                                    QT[:, h, s * STRIPE + qoff:(s + 1) * STRIPE],
                                    start=True, stop=True,
                                )
                                if kt >= 4 * s:  # diagonal tile: causal mask
                                    nc.vector.tensor_add(
                                        sp[:, qoff:qoff + P],
                                        sp[:, qoff:qoff + P], mask_sb[:])
                                pt = ptp.tile([P, STRIPE], dt.bfloat16,
                                              name="pt", tag="pt")
                                nc.scalar.activation(
                                    pt[:, qoff:STRIPE], sp[:, qoff:STRIPE],
                                    AF.Exp, scale=ATTN_SCALE)
                                pts.append(pt)
                            for j in range(4):
                                jq = 4 * s + j
                                yt = yps.tile([P, HD + 1], dt.float32,
                                              name="yt", tag="yt")
                                for kt in range(jq + 1):
                                    nc.tensor.matmul(
                                        yt[:],
                                        pts[kt][:, j * P:(j + 1) * P],
                                        Vb[:, kt, h * (HD + 1):(h + 1) * (HD + 1)],
                                        start=(kt == 0), stop=(kt == jq),
                                    )
                                rec = recp.tile([P, 1], dt.float32,
                                                name="rec", tag="rec")
                                nc.vector.reciprocal(rec[:], yt[:, HD:HD + 1])
                                nc.vector.tensor_scalar_mul(
                                    Y[:, jq, h * HD:(h + 1) * HD],
                                    yt[:, 0:HD], rec[:])
                        # transpose this head's Y -> staging -> DRAM (bf16)
                        yst = ytstp.tile([P, t_len], dt.bfloat16,
                                         name="yst", tag="yst")
                        for tt in range(n_tt):
                            ps = ytps.tile([P, P], dt.float32,
                                           name="ps_y", tag="ytp")
                            nc.tensor.transpose(
                                ps[:], Y[:, tt, h * HD:(h + 1) * HD], ident[:])
                            nc.vector.tensor_copy(
                                yst[:, tt * P:(tt + 1) * P], ps[:])
                        nc.sync.dma_start(ytl_ds[h][:], yst[:])
                        if phases >= 3:
                            # start this head's pair AllGather immediately so it
                            # overlaps the remaining heads' attention
                            nc.gpsimd.collective_compute(
                                "AllGather", ALU.bypass,
                                replica_groups=[[i, i + 1]
                                                for i in range(0, N_CORES, 2)],
                                ins=[ytl_ds[h].opt()], outs=[ytg_ds[h].opt()],
                            )

            rep_loop.close()

            # ====== phase 3: pair AllGather + output projection ======
            if phases == 2:
                with tc.tile_pool(name="dbg2", bufs=1) as dbgp:
                    z = dbgp.tile([P, F], dt.bfloat16)
                    nc.sync.dma_start(z[:], ytl_ds[0][0:P, 0:F])
                    o32 = dbgp.tile([P, F], dt.float32)
                    nc.vector.tensor_copy(o32[:], z[:])
                    for tt in range(n_tt):
                        nc.sync.dma_start(out_d[tt * P:(tt + 1) * P, :], o32[:])
            for _tail in range(reps_tail if phases >= 3 else 0):
                with (
                    tc.tile_pool(name="wpp", bufs=1) as wpp,
                    tc.tile_pool(name="wstg2", bufs=2) as wstg2,
                    tc.tile_pool(name="ytg", bufs=1) as ytgp,
                    tc.tile_pool(name="acc", bufs=1) as accp,
                    tc.tile_pool(name="osb", bufs=3) as osbp,
                    tc.tile_pool(name="cps", bufs=4, space="PSUM") as cps,
                ):
                    WP = wpp.tile([P, 8, F], dt.bfloat16)
                    for fc in range(8):
                        ws = wstg2.tile([P, F], dt.float32, name="ws", tag="ws")
                        nc.sync.dma_start(ws[:], wp_d[:, fc, :])
                        nc.vector.tensor_copy(WP[:, fc, :], ws[:])
                    ACC = accp.tile([P, n_tt, F], dt.float32)
                    # one pass per head-AllGather: the partial products for
                    # head pair (h, 4+h) accumulate into ACC as soon as that
                    # AllGather lands; only the last pass is on the tail.
                    for h in range(HG):
                        yc0 = ytgp.tile([P, t_len], dt.bfloat16,
                                        name="yc0", tag=f"yc{h}")
                        yc1 = ytgp.tile([P, t_len], dt.bfloat16,
                                        name="yc1", tag=f"yd{h}")
                        nc.sync.dma_start(yc0[:], ytg_ds[h][0:P, :])
                        nc.sync.dma_start(yc1[:], ytg_ds[h][P:2 * P, :])
                        for tt in range(n_tt):
                            cp = cps.tile([P, F], dt.float32, name="cp", tag="cp")
                            if h == 0:
                                # fold the bias in via a K=1 ones matmul
                                nc.tensor.matmul(cp[:], ones_bf[:], bias_bf[:],
                                                 start=True, stop=False)
                            nc.tensor.matmul(
                                cp[:], yc0[:, tt * P:(tt + 1) * P], WP[:, h, :],
                                start=(h != 0), stop=False)
                            nc.tensor.matmul(
                                cp[:], yc1[:, tt * P:(tt + 1) * P],
                                WP[:, HG + h, :], start=False, stop=True)
                            if h == 0:
                                nc.scalar.copy(ACC[:, tt, :], cp[:])
                            elif h < HG - 1:
                                nc.vector.tensor_add(ACC[:, tt, :], cp[:],
                                                     ACC[:, tt, :])
                            else:
                                o_sb = osbp.tile([P, F], dt.float32,
                                                 name="o_sb", tag="o")
                                nc.vector.tensor_add(o_sb[:], cp[:], ACC[:, tt, :])
                                nc.sync.dma_start(
                                    out_d[tt * P:(tt + 1) * P, :], o_sb[:])

    nc.compile()
    return nc


_NC_CACHE = {}


def _get_nc(t_len=T):
    if t_len not in _NC_CACHE:
        _NC_CACHE[t_len] = _build_nc(t_len)
    return _NC_CACHE[t_len]


def make_in_maps(x, ve, qkv_w, lambdas, c_proj_w, c_proj_b, t_len=T):
    """Host-side sharding + constant tables. Pure relayout/slicing."""
    x = np.asarray(x, np.float32)
    ve = np.asarray(ve, np.float32)
    qkv_w = np.asarray(qkv_w, np.float32)
    lambdas = np.asarray(lambdas, np.float32)
    c_proj_w = np.asarray(c_proj_w, np.float32)
    c_proj_b = np.asarray(c_proj_b, np.float32)

    n_tt = t_len // P
    half = HD // 2
    inv_freq = (1.0 / (10000.0 ** (np.arange(half, dtype=np.float64) / half)))
    ang = np.arange(t_len, dtype=np.float64)[:, None] * inv_freq[None, :]
    cos = np.cos(ang).astype(np.float32)
    sin = np.sin(ang).astype(np.float32)
    cosr = np.concatenate([cos, cos], axis=1)            # [T, 128]
    sinr = np.concatenate([-sin, sin], axis=1)           # [T, 128]
    cosr = np.ascontiguousarray(cosr.reshape(n_tt, P, HD).transpose(1, 0, 2))
    sinr = np.ascontiguousarray(sinr.reshape(n_tt, P, HD).transpose(1, 0, 2))

    kk, qq = np.meshgrid(np.arange(P), np.arange(P), indexing="ij")
    dmask = np.where(kk <= qq, 0.0, NEG_INF).astype(np.float32)
    lam = np.tile(lambdas.reshape(1, 2), (P, 1)).astype(np.float32)

    in_maps = []
    for c in range(N_CORES):
        b, hg = c // 2, c % 2
        wslc = np.concatenate(
            [qkv_w[e, hg * F:(hg + 1) * F, :] for e in range(3)], axis=0)  # [1536, DIM]
        wq = np.ascontiguousarray(
            wslc.T.reshape(8, P, 3 * F).transpose(1, 0, 2))              # [128, 8, 1536]
        vesl = np.ascontiguousarray(
            ve[b].reshape(t_len, H, HD)[:, hg * HG:(hg + 1) * HG, :].reshape(t_len, F))
        wp = np.ascontiguousarray(
            c_proj_w[hg * F:(hg + 1) * F, :].T.reshape(8, P, F).transpose(1, 0, 2))
        in_maps.append({
            "x": np.ascontiguousarray(x[b]),
            "wq": wq,
            "ve": vesl,
            "lam": lam,
            "cosr": cosr,
            "sinr": sinr,
            "dmask": dmask,
            "wp": wp,
            "bias": c_proj_b[hg * F:(hg + 1) * F].reshape(1, F).copy(),
        })
    return in_maps


def assemble(results):
    """Concatenate per-core output shards into the full [B, T, DIM] output."""
    outs = []
    for b in range(B):
        outs.append(np.concatenate(
            [results[2 * b]["out"], results[2 * b + 1]["out"]], axis=1))
    return np.stack(outs, axis=0)


def kernel(x, ve, qkv_w, lambdas, c_proj_w, c_proj_b):
    nc = _get_nc(T)
    in_maps = make_in_maps(x, ve, qkv_w, lambdas, c_proj_w, c_proj_b, T)
    r = run_bass_kernel_spmd(nc, in_maps, list(range(N_CORES)))
    return assemble(r.results)

